# revision 1
# baseline (speedup 1.0000x reference)
"""Trainium2 Bass kernel for GQA attention (B=2, S=2048, D=2048, H=16, KVH=4).

Sharding: 8 cores = (batch b in {0,1}) x (kv-group g in {0..3}).
Each core: Q/K/V projections for its 4 q-heads + 1 kv head, RoPE, causal
softmax attention, and a partial output projection over its 512 Wo rows.
Host sums the 4 partials per batch.

On-device layout notes:
- x is passed per-core pre-transposed (xT [D, S]) so the contraction dim
  (D, then head_dim, then seq-k) is always the SBUF partition dim.
- Wq/Wk columns are pre-permuted per head on host to deinterleave RoPE
  pairs (even dims -> rows 0:64, odd dims -> rows 64:128 of each head's
  Q^T/K^T block). The same permutation on Q and K preserves q.k dots.
- Scores are computed transposed (S^T [k, q]); the PV matmul consumes the
  exp'd scores directly (O^T = sum_k V[k,:]^T E^T[k,:]) so the probability
  matrix is never transposed. Softmax denominators come from an extra
  matmul with an all-ones [128,128] stationary (result broadcast across
  all partitions), normalized with an approx reciprocal + multiply.
- All big matmuls are fp32r (1 cyc/row at free-size >= 256); fp32r
  operands must be produced as fp32r, so matmul-feeding tiles/DRAM
  tensors are declared float32r (bit-identical to f32 on host).
"""

import math

import numpy as np

B = 2
S = 2048
D = 2048
N_HEADS = 16
N_KV_HEADS = 4
HD = 128  # head dim
G = 4  # kv groups (= heads per core group)
HPC = 4  # q heads per core
EQ = HPC * HD  # 512 q-proj cols per core
THETA = 10000.0
N_CORES = 8

SC = 4  # seq chunks of 512 in projections
QC = 4  # q chunks of 512 in attention
KT = 16  # k tiles of 128
DT = 16  # d tiles of 128


def _host_tables():
    """cos/sin tables [64, S] (transposed), deinterleave permutation [128]."""
    j = np.arange(HD // 2)
    inv_freq = 1.0 / THETA ** (2 * j / HD)  # [64]
    t = np.arange(S)
    ang = np.outer(inv_freq, t)  # [64, S]
    cosT = np.cos(ang).astype(np.float32)
    sinT = np.sin(ang).astype(np.float32)
    jj = np.arange(HD)
    perm = np.where(jj < 64, 2 * jj, 2 * (jj - 64) + 1)  # new row j <- old dim perm[j]
    return cosT, sinT, perm


def _host_masks():
    """Diagonal-subtile mask [128,128]: 1 where qq >= kk (causal-inclusive)."""
    kk = np.arange(128)[:, None]
    qq = np.arange(128)[None, :]
    return (qq >= kk).astype(np.float32)


def _emit_once(nc, tc, mybir, aps, shared, trim=True, split=True):
    """One full forward pass (phases B: proj+RoPE+V, C: attention, D: out-proj)."""
    f32 = mybir.dt.float32
    f32r = mybir.dt.float32r
    AT = mybir.ActivationFunctionType
    xT, wq, wk, wv, wo, cosT, sinT, out = aps
    masks_sb, ones_sb, ident, qrot, krot, vsb = shared
    inv_sqrt_hd = 1.0 / math.sqrt(HD)

    mpsum_ctx = tc.tile_pool(name="mpsum", bufs=1, space="PSUM")
    mpsum = mpsum_ctx.__enter__()

    # ---------------- Phase B: projections + RoPE + V ----------------
    with (
        tc.tile_pool(name="wpool", bufs=1) as wpool,
        tc.tile_pool(name="cspool", bufs=1) as cspool,
        tc.tile_pool(name="xs", bufs=24) as xs_pool,
        tc.tile_pool(name="ropetmp", bufs=2) as tmp_pool,
        tc.tile_pool(name="vtstage", bufs=1) as vt_pool,
    ):
        wq_sb = wpool.tile([128, DT, EQ], f32r, tag="wq")
        wk_sb = wpool.tile([128, DT, HD], f32r, tag="wk")
        wv_sb = wpool.tile([128, DT, HD], f32r, tag="wv")
        # interleave the first seq-chunk's activation loads with the weight
        # slices so the d=0 matmuls un-gate early on a cold start; one DMA
        # instruction = one HW queue, so splitting also parallelizes.
        # tiny first loads so the very first matmul (kp, d=0) ungates in
        # ~2-3us: wk d=0 slice and the two halves of x chunk0/d0 go first
        xts0 = []
        xt00 = xs_pool.tile([128, 512], f32r, tag="xs", name="xs0_0")
        nc.sync.dma_start(wk_sb[:, 0, :], wk[0:128, :])
        nc.sync.dma_start(xt00[:, 0:256], xT[0:128, 0:256])
        nc.sync.dma_start(xt00[:, 256:512], xT[0:128, 256:512])
        xts0.append(xt00)
        wkr = wk.rearrange("(t p) e -> p t e", p=128)
        wvr = wv.rearrange("(t p) e -> p t e", p=128)
        nc.sync.dma_start(wk_sb[:, 1:4, :], wkr[:, 1:4, :])
        for d in range(DT):
            if d > 0:
                xt = xs_pool.tile([128, 512], f32r, tag="xs", name=f"xs0_{d}")
                nc.sync.dma_start(xt[:], xT[d * 128 : (d + 1) * 128, 0:512])
                xts0.append(xt)
            nc.sync.dma_start(wq_sb[:, d, :], wq[d * 128 : (d + 1) * 128, :])
            if d % 4 == 0:
                if d > 0:
                    nc.sync.dma_start(wk_sb[:, d : d + 4, :], wkr[:, d : d + 4, :])
                nc.sync.dma_start(wv_sb[:, d : d + 4, :], wvr[:, d : d + 4, :])
        cos_sb = cspool.tile([64, S], f32, tag="cos")
        sin_sb = cspool.tile([64, S], f32, tag="sin")
        nc.sync.dma_start(cos_sb[:], cosT)
        nc.sync.dma_start(sin_sb[:], sinT)
        vT_sb = vt_pool.tile([128, S], f32, tag="vT")

        for sc in range(SC):
            ssl = slice(sc * 512, (sc + 1) * 512)
            if sc == 0:
                xts = xts0
            else:
                xts = []
                for d in range(DT):
                    xt = xs_pool.tile([128, 512], f32r, tag="xs", name=f"xs{sc}_{d}")
                    nc.sync.dma_start(xt[:], xT[d * 128 : (d + 1) * 128, ssl])
                    xts.append(xt)
            # Per-target accumulation loops (kp, qp0..qp3, vp) with RoPE
            # emitted right after each target completes: the RoPE pipeline
            # runs ~one target behind the matmuls instead of all landing
            # after the chunk, which shrinks the projection->attention
            # seam (the attention PSUM pools wait on this pool's releases).
            c = cos_sb[:, ssl]
            s = sin_sb[:, ssl]

            def _rope(src, dst):
                t1 = tmp_pool.tile([64, 512], f32, tag="t1", name="t1")
                t2 = tmp_pool.tile([64, 512], f32, tag="t2", name="t2")
                t3 = tmp_pool.tile([64, 512], f32, tag="t3", name="t3")
                t4 = tmp_pool.tile([64, 512], f32, tag="t4", name="t4")
                nc.vector.tensor_mul(t1[:], src[0:64, :], c)
                nc.vector.tensor_mul(t2[:], src[64:128, :], s)
                nc.vector.tensor_mul(t3[:], src[0:64, :], s)
                nc.vector.tensor_mul(t4[:], src[64:128, :], c)
                nc.vector.tensor_sub(dst[0:64, :], t1[:], t2[:])
                nc.vector.tensor_add(dst[64:128, :], t3[:], t4[:])

            kp = mpsum.tile([128, 512], f32, tag="kv", bufs=2, name="kp")
            for d in range(DT):
                if sc == 0 and d == 0:
                    nc.tensor.matmul(
                        kp[:, 0:256], wk_sb[:, 0, :], xts[0][:, 0:256],
                        start=True, stop=False,
                    )
                    nc.tensor.matmul(
                        kp[:, 256:512], wk_sb[:, 0, :], xts[0][:, 256:512],
                        start=False, stop=False,
                    )
                else:
                    nc.tensor.matmul(
                        kp[:], wk_sb[:, d, :], xts[d][:],
                        start=d == 0, stop=d == DT - 1,
                    )
            _rope(kp, krot[sc])
            vp = mpsum.tile([128, 512], f32, tag="kv", bufs=2, name="vp")
            for d in range(DT):
                nc.tensor.matmul(
                    vp[:], wv_sb[:, d, :], xts[d][:], start=d == 0, stop=d == DT - 1
                )
            # V^T evac (ScalarE keeps DVE free), then transpose this
            # chunk's four V tiles right away so PE work stays spread out
            nc.scalar.copy(vT_sb[:, ssl], vp[:])
            for t in range(4 * sc, 4 * sc + 4):
                vtp = mpsum.tile([128, 128], f32, tag="vtr", bufs=2, name="vtr")
                nc.tensor.transpose(
                    vtp[:], vT_sb[:, t * 128 : (t + 1) * 128], ident[:]
                )
                nc.scalar.copy(vsb[:, t, :], vtp[:])
            for h in range(HPC):
                qp = mpsum.tile(
                    [128, 512], f32, tag="qp", bufs=4, name=f"qp{h}"
                )
                for d in range(DT):
                    nc.tensor.matmul(
                        qp[:],
                        wq_sb[:, d, h * 128 : (h + 1) * 128],
                        xts[d][:],
                        start=d == 0,
                        stop=d == DT - 1,
                    )
                _rope(qp, qrot[(h, sc)])

    # ---------------- Phase C: attention per (head, q-chunk) --------
    with (
        tc.tile_pool(name="opool", bufs=1) as opool,
        tc.tile_pool(name="wopool", bufs=1) as wopool,
    ):
        orot = {
            (h, c): opool.tile(
                [128, 512], f32r, tag=f"orot{h}_{c}", name=f"orot{h}_{c}"
            )
            for h in range(HPC)
            for c in range(QC)
        }
        wo_sb = wopool.tile([128, HPC, D], f32r, tag="wo")
        for h in range(HPC):
            nc.sync.dma_start(wo_sb[:, h, :], wo[h * 128 : (h + 1) * 128, :])

        with (
            tc.tile_pool(name="et", bufs=12) as et_pool,
            tc.tile_pool(name="rd", bufs=2) as rd_pool,
        ):
            for h in range(HPC):
                for qc in range(QC):
                    qsl = slice(qc * 512, (qc + 1) * 512)
                    nkt = 4 * (qc + 1)
                    op = mpsum.tile([128, 512], f32, tag="vtr", bufs=2, name="op")
                    dp = mpsum.tile([128, 512], f32, tag="kv", bufs=2, name="dp")
                    for kt in range(nkt):
                        # causal trim: diagonal k-tile (r>=0) only produces
                        # q-columns >= 128*r of this chunk; skip the rest.
                        r = kt - 4 * qc
                        lo = 128 * r if (trim and r > 0) else 0
                        w = slice(lo, 512)
                        sp = mpsum.tile([128, 512], f32, tag="qp", bufs=4, name="sp")
                        kc, ko = divmod(kt, 4)
                        nc.tensor.matmul(
                            sp[:, w],
                            krot[kc][:, ko * 128 : (ko + 1) * 128],
                            qrot[(h, qc)][:, lo:512],
                            start=True,
                            stop=True,
                        )
                        et = et_pool.tile([128, 512], f32r, tag="et", name="et")
                        nc.scalar.activation(et[:, w], sp[:, w], AT.Exp, scale=inv_sqrt_hd)
                        if r >= 0:
                            # mask the [128,128] diagonal subtile only
                            nc.vector.tensor_mul(
                                et[:, lo : lo + 128],
                                et[:, lo : lo + 128],
                                masks_sb[:],
                            )
                        st = kt == 0
                        spf = kt == nkt - 1
                        nc.tensor.matmul(
                            op[:, w], vsb[:, kt, :], et[:, w], start=st, stop=spf
                        )
                        nc.tensor.matmul(
                            dp[:, w], ones_sb[:], et[:, w], start=st, stop=spf
                        )
                    rd = rd_pool.tile([128, 512], f32, tag="rd", name="rd")
                    nc.vector.reciprocal_approx_fast(rd[:], dp[:])
                    nc.vector.tensor_mul(orot[(h, qc)][:], op[:], rd[:])

        # ---------------- Phase D: output projection ----------------
        with (
            tc.tile_pool(name="ostage", bufs=6) as ostage_pool,
        ):
            for st in range(16):
                stsl = slice(st * 128, (st + 1) * 128)
                for mc in range(4):
                    msl = slice(mc * 512, (mc + 1) * 512)
                    pout = mpsum.tile([128, 512], f32, tag="qp", bufs=4, name="pout")
                    sc_, so = divmod(st, 4)
                    for h in range(HPC):
                        nc.tensor.matmul(
                            pout[:],
                            orot[(h, sc_)][:, so * 128 : (so + 1) * 128],
                            wo_sb[:, h, msl],
                            start=(h == 0),
                            stop=(h == HPC - 1),
                        )
                    ost = ostage_pool.tile([128, 512], f32, tag="ost", name="ost")
                    nc.scalar.copy(ost[:], pout[:])
                    nc.sync.dma_start(out[stsl, msl], ost[:])

    mpsum_ctx.__exit__(None, None, None)


def _build_program(reps: int = 1, trim: bool = True, split: bool = True):
    import concourse.mybir as mybir
    import concourse.tile as tile
    from concourse import bacc
    from concourse.masks import make_identity

    f32 = mybir.dt.float32
    f32r = mybir.dt.float32r

    nc = bacc.Bacc(
        "TRN2",
        target_bir_lowering=False,
        debug=False,
        enable_asserts=True,
        num_devices=N_CORES,
    )

    xT = nc.dram_tensor("xT", [D, S], f32r, kind="ExternalInput").ap()
    wq = nc.dram_tensor("wq", [D, EQ], f32r, kind="ExternalInput").ap()
    wk = nc.dram_tensor("wk", [D, HD], f32r, kind="ExternalInput").ap()
    wv = nc.dram_tensor("wv", [D, HD], f32r, kind="ExternalInput").ap()
    wo = nc.dram_tensor("wo", [EQ, D], f32r, kind="ExternalInput").ap()
    cosT = nc.dram_tensor("cosT", [64, S], f32, kind="ExternalInput").ap()
    sinT = nc.dram_tensor("sinT", [64, S], f32, kind="ExternalInput").ap()
    masks4 = nc.dram_tensor("masks4", [128, 128], f32, kind="ExternalInput").ap()
    out = nc.dram_tensor("out", [S, D], f32, kind="ExternalOutput").ap()
    aps = (xT, wq, wk, wv, wo, cosT, sinT, out)

    with tile.TileContext(nc) as tc:
        with (
            tc.tile_pool(name="persist", bufs=1) as persist,
            tc.tile_pool(name="consts", bufs=1) as consts,
        ):
            qrot = {
                (h, c): persist.tile(
                    [128, 512], f32r, tag=f"qrot{h}_{c}", name=f"qrot{h}_{c}"
                )
                for h in range(HPC)
                for c in range(SC)
            }
            krot = {
                c: persist.tile([128, 512], f32r, tag=f"krot{c}", name=f"krot{c}")
                for c in range(SC)
            }
            vsb = persist.tile([128, KT, HD], f32r, tag="vsb")

            masks_sb = consts.tile([128, 128], f32, tag="masks")
            nc.sync.dma_start(masks_sb[:], masks4)
            ones_f32 = consts.tile([128, 128], f32, tag="ones_f32")
            nc.gpsimd.memset(ones_f32[:], 1.0)
            ones_sb = consts.tile([128, 128], f32r, tag="ones")
            nc.vector.tensor_copy(ones_sb[:], ones_f32[:])
            ident = consts.tile([128, 128], f32, tag="ident")
            make_identity(nc, ident[:])

            shared = (masks_sb, ones_sb, ident, qrot, krot, vsb)
            for _rep in range(reps):
                _emit_once(nc, tc, mybir, aps, shared, trim=trim, split=split)

    nc.compile()
    return nc


def _make_in_maps(x, Wq, Wk, Wv, Wo):
    cosT, sinT, perm = _host_tables()
    masks4 = _host_masks()
    x = np.asarray(x, np.float32)
    Wq = np.asarray(Wq, np.float32)
    Wk = np.asarray(Wk, np.float32)
    Wv = np.asarray(Wv, np.float32)
    Wo = np.asarray(Wo, np.float32)

    # per-head column deinterleave for RoPE half-form
    qperm = np.concatenate([h * HD + perm for h in range(N_HEADS)])
    kperm = np.concatenate([h * HD + perm for h in range(N_KV_HEADS)])
    Wqp = Wq[:, qperm]
    Wkp = Wk[:, kperm]

    in_maps = []
    for core in range(N_CORES):
        b, g = divmod(core, G)
        in_maps.append(
            {
                "xT": np.ascontiguousarray(x[b].T),
                "wq": np.ascontiguousarray(Wqp[:, g * EQ : (g + 1) * EQ]),
                "wk": np.ascontiguousarray(Wkp[:, g * HD : (g + 1) * HD]),
                "wv": np.ascontiguousarray(Wv[:, g * HD : (g + 1) * HD]),
                "wo": np.ascontiguousarray(Wo[g * EQ : (g + 1) * EQ, :]),
                "cosT": cosT,
                "sinT": sinT,
                "masks4": masks4,
            }
        )
    return in_maps


_CACHE = {}


def _get_program(reps: int = 1, trim: bool = True, split: bool = True):
    key = ("nc", reps, trim, split)
    if key not in _CACHE:
        _CACHE[key] = _build_program(reps, trim=trim, split=split)
    return _CACHE[key]


def kernel(x, mask, Wq, Wk, Wv, Wo):
    from concourse.bass_utils import run_bass_kernel_spmd

    nc = _get_program()
    in_maps = _make_in_maps(x, Wq, Wk, Wv, Wo)
    res = run_bass_kernel_spmd(nc, in_maps, core_ids=list(range(N_CORES)))
    parts = [res.results[c]["out"] for c in range(N_CORES)]
    out = np.stack(
        [
            parts[0] + parts[1] + parts[2] + parts[3],
            parts[4] + parts[5] + parts[6] + parts[7],
        ]
    ).astype(np.float32)
    return out



# revision 11
# speedup vs baseline: 1.9820x; 1.9820x over previous
"""Trainium2 Bass kernel for GQA attention (B=2, S=2048, D=2048, H=16, KVH=4).

Sharding: 8 cores = (batch b in {0,1}) x (kv-group g in {0..3}).
Each core: Q/K/V projections for its 4 q-heads + 1 kv head, RoPE, causal
softmax attention, and a partial output projection over its 512 Wo rows.
Host sums the 4 partials per batch.

v2 layout notes (vs the fp32r v1):
- All matmul operands are bf16 (PSUM accumulation stays fp32): halves
  HBM traffic and SBUF footprint, makes the causal trim fully effective
  (bf16 matmul is 1 cyc/row at any free size; fp32r pays 4x below 256),
  and unlocks the DVE 2-byte fast modes.
- RoPE: the scalar engine evacuates each Q/K projection PSUM tile to a
  bf16 SBUF staging tile, then the 6 DVE tensor ops run all-bf16 from
  SBUF (4x DVE mode) instead of fp32 reads from PSUM (~4x faster, and
  the PSUM bank frees earlier).
- Scores are computed transposed (S^T [k, q]); the PV matmul consumes the
  exp'd scores directly; softmax denominators via an all-ones stationary
  matmul accumulated over k-tiles; normalized with approx reciprocal.
- Input/weight/output DMAs are issued from the (otherwise idle) gpsimd
  queue: Pool-engine DMA issue is ~36ns vs ~565ns on sync/SP, so the
  x-tile stream is not serialized behind slow issue.
- out is written bf16; the host converts to f32 and sums the 4 partials.
"""

import math

import numpy as np

B = 2
S = 2048
D = 2048
N_HEADS = 16
N_KV_HEADS = 4
HD = 128  # head dim
G = 4  # kv groups (= heads per core group)
HPC = 4  # q heads per core
EQ = HPC * HD  # 512 q-proj cols per core
THETA = 10000.0
N_CORES = 8

SC = 4  # seq chunks of 512 in projections
QC = 4  # q chunks of 512 in attention
KT = 16  # k tiles of 128
DT = 16  # d tiles of 128


def _host_tables():
    """cos/sin tables [128, S] (transposed, duplicated across partition
    halves so DVE tensor_tensor reads have matching base partitions),
    deinterleave permutation [128]."""
    j = np.arange(HD // 2)
    inv_freq = 1.0 / THETA ** (2 * j / HD)  # [64]
    t = np.arange(S)
    ang = np.outer(inv_freq, t)  # [64, S]
    cosT = np.cos(ang).astype(np.float32)
    sinT = np.sin(ang).astype(np.float32)
    cosT = np.concatenate([cosT, cosT], axis=0)  # [128, S]
    sinT = np.concatenate([sinT, sinT], axis=0)
    jj = np.arange(HD)
    perm = np.where(jj < 64, 2 * jj, 2 * (jj - 64) + 1)  # new row j <- old dim perm[j]
    return cosT, sinT, perm


def _host_masks():
    """Diagonal-subtile mask [128,128]: 1 where qq >= kk (causal-inclusive)."""
    kk = np.arange(128)[:, None]
    qq = np.arange(128)[None, :]
    return (qq >= kk).astype(np.float32)


def _emit_once(nc, tc, mybir, aps, shared, trim=True, split=True):
    """One full forward pass (phases B: proj+RoPE+V, C: attention, D: out-proj)."""
    f32 = mybir.dt.float32
    bf16 = mybir.dt.bfloat16
    AT = mybir.ActivationFunctionType
    xT, wq, wk, wv, wo, cosT, sinT, out = aps
    masks_sb, ones_sb, ident, qrot, krot, vsb = shared
    inv_sqrt_hd = 1.0 / math.sqrt(HD)

    mpsum_ctx = tc.tile_pool(name="mpsum", bufs=1, space="PSUM")
    mpsum = mpsum_ctx.__enter__()

    # ---------------- Phase B: projections + RoPE + V ----------------
    with (
        tc.tile_pool(name="wpool", bufs=1) as wpool,
        tc.tile_pool(name="cspool", bufs=1) as cspool,
        tc.tile_pool(name="xs", bufs=32) as xs_pool,
        tc.tile_pool(name="ropetmp", bufs=2) as tmp_pool,
        tc.tile_pool(name="vtstage", bufs=1) as vt_pool,
    ):
        wq_sb = wpool.tile([128, DT, EQ], bf16, tag="wq")
        wk_sb = wpool.tile([128, DT, HD], bf16, tag="wk")
        wv_sb = wpool.tile([128, DT, HD], bf16, tag="wv")
        # interleave the first seq-chunk's activation loads with the weight
        # slices so the d=0 matmuls un-gate early on a cold start; one DMA
        # instruction = one HW queue, so splitting also parallelizes.
        # tiny first loads so the very first matmul (kp, d=0) ungates in
        # ~2-3us: wk d=0 slice and the two halves of x chunk0/d0 go first
        # x-tile loads ride the gpsimd queue; weight/table loads ride the
        # sync (SP) queue so neither issue stream serializes the other.
        # Chunk-0 x tiles are split in half so the first kp accumulation
        # chain isn't gated on full-tile DMA latency.
        xts0 = []
        wkr = wk.rearrange("(t p) e -> p t e", p=128)
        wvr = wv.rearrange("(t p) e -> p t e", p=128)
        nc.sync.dma_start(wk_sb[:, 0, :], wk[0:128, :])
        for d in range(DT):
            xt = xs_pool.tile([128, 512], bf16, tag="xs", name=f"xs0_{d}")
            nc.gpsimd.dma_start(xt[:, 0:256], xT[d * 128 : (d + 1) * 128, 0:256])
            nc.gpsimd.dma_start(
                xt[:, 256:512], xT[d * 128 : (d + 1) * 128, 256:512]
            )
            xts0.append(xt)
            if d == 0:
                nc.sync.dma_start(wk_sb[:, 1:4, :], wkr[:, 1:4, :])
            if d % 4 == 0:
                if d > 0:
                    nc.sync.dma_start(wk_sb[:, d : d + 4, :], wkr[:, d : d + 4, :])
                nc.sync.dma_start(wv_sb[:, d : d + 4, :], wvr[:, d : d + 4, :])
        for d in range(DT):
            nc.sync.dma_start(wq_sb[:, d, :], wq[d * 128 : (d + 1) * 128, :])
        cos_sb = cspool.tile([128, S], bf16, tag="cos")
        sin_sb = cspool.tile([128, S], bf16, tag="sin")
        nc.sync.dma_start(cos_sb[:], cosT)
        nc.sync.dma_start(sin_sb[:], sinT)
        vT_sb = vt_pool.tile([128, S], bf16, tag="vT")

        for sc in range(SC):
            ssl = slice(sc * 512, (sc + 1) * 512)
            if sc == 0:
                xts = xts0
            else:
                xts = []
                for d in range(DT):
                    xt = xs_pool.tile([128, 512], bf16, tag="xs", name=f"xs{sc}_{d}")
                    nc.gpsimd.dma_start(xt[:], xT[d * 128 : (d + 1) * 128, ssl])
                    xts.append(xt)
            # Per-target accumulation loops (kp, qp0..qp3, vp) with RoPE
            # emitted right after each target completes: the RoPE pipeline
            # runs ~one target behind the matmuls instead of all landing
            # after the chunk, which shrinks the projection->attention
            # seam (the attention PSUM pools wait on this pool's releases).
            c_lo = cos_sb[0:64, ssl]
            c_hi = cos_sb[64:128, ssl]
            s_lo = sin_sb[0:64, ssl]
            s_hi = sin_sb[64:128, ssl]

            def _rope(src_psum, dst, nm):
                # scalar evacuates PSUM f32 -> bf16 SBUF (frees the bank),
                # then all-bf16 SBUF DVE ops run in 4x mode; the cos/sin
                # tables are duplicated across partition halves so both
                # SBUF inputs of each mul share a base partition
                xsrc = tmp_pool.tile([128, 512], bf16, tag="xsrc", name=f"xsrc{nm}")
                nc.scalar.copy(xsrc[:], src_psum[:])
                t1 = tmp_pool.tile([64, 512], bf16, tag="t1", name="t1")
                t2 = tmp_pool.tile([64, 512], bf16, tag="t2", name="t2")
                t3 = tmp_pool.tile([64, 512], bf16, tag="t3", name="t3")
                t4 = tmp_pool.tile([64, 512], bf16, tag="t4", name="t4")
                nc.vector.tensor_mul(t1[:], xsrc[0:64, :], c_lo)
                nc.vector.tensor_mul(t2[:], xsrc[64:128, :], s_hi)
                nc.vector.tensor_mul(t3[:], xsrc[0:64, :], s_lo)
                nc.vector.tensor_mul(t4[:], xsrc[64:128, :], c_hi)
                nc.vector.tensor_sub(dst[0:64, :], t1[:], t2[:])
                nc.vector.tensor_add(dst[64:128, :], t3[:], t4[:])

            kp = mpsum.tile([128, 512], f32, tag="kv", bufs=2, name="kp")
            for d in range(DT):
                if sc == 0 and d == 0:
                    nc.tensor.matmul(
                        kp[:, 0:256], wk_sb[:, 0, :], xts[0][:, 0:256],
                        start=True, stop=False,
                    )
                    nc.tensor.matmul(
                        kp[:, 256:512], wk_sb[:, 0, :], xts[0][:, 256:512],
                        start=False, stop=False,
                    )
                else:
                    nc.tensor.matmul(
                        kp[:], wk_sb[:, d, :], xts[d][:],
                        start=d == 0, stop=d == DT - 1,
                    )
            _rope(kp, krot[sc], f"k{sc}")
            vp = mpsum.tile([128, 512], f32, tag="kv", bufs=2, name="vp")
            for d in range(DT):
                nc.tensor.matmul(
                    vp[:], wv_sb[:, d, :], xts[d][:], start=d == 0, stop=d == DT - 1
                )
            # V^T evac (ScalarE keeps DVE free), then transpose this
            # chunk's four V tiles right away so PE work stays spread out
            nc.scalar.copy(vT_sb[:, ssl], vp[:])
            for t in range(4 * sc, 4 * sc + 4):
                vtp = mpsum.tile([128, 128], bf16, tag="vtr", bufs=2, name="vtr")
                nc.tensor.transpose(
                    vtp[:], vT_sb[:, t * 128 : (t + 1) * 128], ident[:]
                )
                nc.scalar.copy(vsb[:, t, :], vtp[:])
            for h in range(HPC):
                qp = mpsum.tile(
                    [128, 512], f32, tag="qp", bufs=4, name=f"qp{h}"
                )
                for d in range(DT):
                    nc.tensor.matmul(
                        qp[:],
                        wq_sb[:, d, h * 128 : (h + 1) * 128],
                        xts[d][:],
                        start=d == 0,
                        stop=d == DT - 1,
                    )
                _rope(qp, qrot[(h, sc)], f"q{h}_{sc}")

    # ---------------- Phase C: attention per (head, q-chunk) --------
    with (
        tc.tile_pool(name="opool", bufs=1) as opool,
        tc.tile_pool(name="wopool", bufs=1) as wopool,
    ):
        orot = {
            (h, c): opool.tile(
                [128, 512], bf16, tag=f"orot{h}_{c}", name=f"orot{h}_{c}"
            )
            for h in range(HPC)
            for c in range(QC)
        }
        wo_sb = wopool.tile([128, HPC, D], bf16, tag="wo")
        for h in range(HPC):
            nc.gpsimd.dma_start(wo_sb[:, h, :], wo[h * 128 : (h + 1) * 128, :])

        with (
            tc.tile_pool(name="et", bufs=12) as et_pool,
            tc.tile_pool(name="rd", bufs=2) as rd_pool,
        ):
            # qc order [1,2,3,0]: the final attention block (and the
            # recip/normalize chain behind it) is the short qc=0 one, and
            # phase D's last-emitted pout group (chunk 0) unblocks early,
            # so PE doesn't idle at the C->D tail.
            for h in range(HPC):
                for qc in (1, 2, 3, 0):
                    qsl = slice(qc * 512, (qc + 1) * 512)
                    nkt = 4 * (qc + 1)
                    op = mpsum.tile([128, 512], f32, tag="vtr", bufs=2, name="op")
                    dp = mpsum.tile([128, 512], f32, tag="kv", bufs=2, name="dp")
                    for kt in range(nkt):
                        # causal trim: diagonal k-tile (r>=0) only produces
                        # q-columns >= 128*r of this chunk; skip the rest.
                        r = kt - 4 * qc
                        lo = 128 * r if (trim and r > 0) else 0
                        w = slice(lo, 512)
                        sp = mpsum.tile([128, 512], f32, tag="qp", bufs=4, name="sp")
                        kc, ko = divmod(kt, 4)
                        nc.tensor.matmul(
                            sp[:, w],
                            krot[kc][:, ko * 128 : (ko + 1) * 128],
                            qrot[(h, qc)][:, lo:512],
                            start=True,
                            stop=True,
                        )
                        et = et_pool.tile([128, 512], bf16, tag="et", name="et")
                        nc.scalar.activation(et[:, w], sp[:, w], AT.Exp, scale=inv_sqrt_hd)
                        if r >= 0:
                            # mask the [128,128] diagonal subtile only
                            nc.vector.tensor_mul(
                                et[:, lo : lo + 128],
                                et[:, lo : lo + 128],
                                masks_sb[:],
                            )
                        st = kt == 0
                        spf = kt == nkt - 1
                        nc.tensor.matmul(
                            op[:, w], vsb[:, kt, :], et[:, w], start=st, stop=spf
                        )
                        nc.tensor.matmul(
                            dp[:, w], ones_sb[:], et[:, w], start=st, stop=spf
                        )
                    rd = rd_pool.tile([128, 512], f32, tag="rd", name="rd")
                    nc.vector.reciprocal_approx_fast(rd[:], dp[:])
                    nc.vector.tensor_mul(orot[(h, qc)][:], op[:], rd[:])

        # ---------------- Phase D: output projection ----------------
        with (
            tc.tile_pool(name="ostage", bufs=6) as ostage_pool,
        ):
            for sc_ in (1, 2, 3, 0):
              for so in range(4):
                st = 4 * sc_ + so
                stsl = slice(st * 128, (st + 1) * 128)
                for mc in range(4):
                    msl = slice(mc * 512, (mc + 1) * 512)
                    pout = mpsum.tile([128, 512], f32, tag="qp", bufs=4, name="pout")
                    for h in range(HPC):
                        nc.tensor.matmul(
                            pout[:],
                            orot[(h, sc_)][:, so * 128 : (so + 1) * 128],
                            wo_sb[:, h, msl],
                            start=(h == 0),
                            stop=(h == HPC - 1),
                        )
                    ost = ostage_pool.tile([128, 512], bf16, tag="ost", name="ost")
                    nc.scalar.copy(ost[:], pout[:])
                    nc.gpsimd.dma_start(out[stsl, msl], ost[:])

    mpsum_ctx.__exit__(None, None, None)


def _build_program(reps: int = 1, trim: bool = True, split: bool = True):
    import concourse.mybir as mybir
    import concourse.tile as tile
    from concourse import bacc
    from concourse.masks import make_identity

    f32 = mybir.dt.float32
    bf16 = mybir.dt.bfloat16

    nc = bacc.Bacc(
        "TRN2",
        target_bir_lowering=False,
        debug=False,
        enable_asserts=True,
        num_devices=N_CORES,
    )

    xT = nc.dram_tensor("xT", [D, S], bf16, kind="ExternalInput").ap()
    wq = nc.dram_tensor("wq", [D, EQ], bf16, kind="ExternalInput").ap()
    wk = nc.dram_tensor("wk", [D, HD], bf16, kind="ExternalInput").ap()
    wv = nc.dram_tensor("wv", [D, HD], bf16, kind="ExternalInput").ap()
    wo = nc.dram_tensor("wo", [EQ, D], bf16, kind="ExternalInput").ap()
    cosT = nc.dram_tensor("cosT", [128, S], bf16, kind="ExternalInput").ap()
    sinT = nc.dram_tensor("sinT", [128, S], bf16, kind="ExternalInput").ap()
    masks4 = nc.dram_tensor("masks4", [128, 128], bf16, kind="ExternalInput").ap()
    out = nc.dram_tensor("out", [S, D], bf16, kind="ExternalOutput").ap()
    aps = (xT, wq, wk, wv, wo, cosT, sinT, out)

    with tile.TileContext(nc) as tc:
        with (
            tc.tile_pool(name="persist", bufs=1) as persist,
            tc.tile_pool(name="consts", bufs=1) as consts,
        ):
            qrot = {
                (h, c): persist.tile(
                    [128, 512], bf16, tag=f"qrot{h}_{c}", name=f"qrot{h}_{c}"
                )
                for h in range(HPC)
                for c in range(SC)
            }
            krot = {
                c: persist.tile([128, 512], bf16, tag=f"krot{c}", name=f"krot{c}")
                for c in range(SC)
            }
            vsb = persist.tile([128, KT, HD], bf16, tag="vsb")

            masks_sb = consts.tile([128, 128], bf16, tag="masks")
            nc.gpsimd.dma_start(masks_sb[:], masks4)
            ones_sb = consts.tile([128, 128], bf16, tag="ones")
            nc.gpsimd.memset(ones_sb[:], 1.0)
            ident = consts.tile([128, 128], bf16, tag="ident")
            make_identity(nc, ident[:])

            shared = (masks_sb, ones_sb, ident, qrot, krot, vsb)
            for _rep in range(reps):
                _emit_once(nc, tc, mybir, aps, shared, trim=trim, split=split)

    nc.compile()
    return nc


def _make_in_maps(x, Wq, Wk, Wv, Wo):
    import ml_dtypes

    bf16 = ml_dtypes.bfloat16
    cosT, sinT, perm = _host_tables()
    masks4 = _host_masks()
    x = np.asarray(x, np.float32)
    Wq = np.asarray(Wq, np.float32)
    Wk = np.asarray(Wk, np.float32)
    Wv = np.asarray(Wv, np.float32)
    Wo = np.asarray(Wo, np.float32)

    # per-head column deinterleave for RoPE half-form
    qperm = np.concatenate([h * HD + perm for h in range(N_HEADS)])
    kperm = np.concatenate([h * HD + perm for h in range(N_KV_HEADS)])
    Wqp = Wq[:, qperm]
    Wkp = Wk[:, kperm]

    in_maps = []
    for core in range(N_CORES):
        b, g = divmod(core, G)
        in_maps.append(
            {
                "xT": np.ascontiguousarray(x[b].T).astype(bf16),
                "wq": np.ascontiguousarray(
                    Wqp[:, g * EQ : (g + 1) * EQ]
                ).astype(bf16),
                "wk": np.ascontiguousarray(
                    Wkp[:, g * HD : (g + 1) * HD]
                ).astype(bf16),
                "wv": np.ascontiguousarray(
                    Wv[:, g * HD : (g + 1) * HD]
                ).astype(bf16),
                "wo": np.ascontiguousarray(Wo[g * EQ : (g + 1) * EQ, :]).astype(bf16),
                "cosT": cosT.astype(bf16),
                "sinT": sinT.astype(bf16),
                "masks4": masks4.astype(bf16),
            }
        )
    return in_maps


_CACHE = {}


def _get_program(reps: int = 1, trim: bool = True, split: bool = True):
    key = ("nc", reps, trim, split)
    if key not in _CACHE:
        _CACHE[key] = _build_program(reps, trim=trim, split=split)
    return _CACHE[key]


def kernel(x, mask, Wq, Wk, Wv, Wo):
    from concourse.bass_utils import run_bass_kernel_spmd

    nc = _get_program()
    in_maps = _make_in_maps(x, Wq, Wk, Wv, Wo)
    res = run_bass_kernel_spmd(nc, in_maps, core_ids=list(range(N_CORES)))
    parts = [res.results[c]["out"].astype(np.float32) for c in range(N_CORES)]
    out = np.stack(
        [
            parts[0] + parts[1] + parts[2] + parts[3],
            parts[4] + parts[5] + parts[6] + parts[7],
        ]
    ).astype(np.float32)
    return out
